# revision 1
# baseline (speedup 1.0000x reference)
"""bf16 C4 depthwise conv v3: PE taps + ACT products + DVE tensor_tensor adds.

Rotations 0,1 (and k2 taps of rot 2) run on the PE as diag-stationary PSUM
matmuls with ACT drains. The 9 base-weight products p_m = w_m (.) x are
computed once per 16-row group by ACT (scale-copy) and shared by rotations
2 and 3, whose remaining taps are plain bf16 tensor_tensor adds on DVE
(2x mode) of shifted product views.
"""

import numpy as np
from contextlib import ExitStack

from concourse import bacc, mybir, tile

B, C, H, W = 16, 192, 128, 128
NCORES = 8
BS = B // NCORES
ROWS = BS * C               # 384
NCHUNK = ROWS // 128        # 3
HT = 32
NHT = H // HT               # 4
SUB = 4
NSUB = HT // SUB            # 8
PH = 16                     # product-group rows
NPG = HT // PH              # 2
TW = W + 2                  # 130
TH = H + 2                  # 130

F32 = mybir.dt.float32
BF16 = mybir.dt.bfloat16

TAPS = [(1, 1)] + [(ti, tj) for ti in range(3) for tj in range(3) if (ti, tj) != (1, 1)]

# VMAP[r][i*3+j] = base-value flat index m sitting at position (i,j) of rot r
_idx = np.arange(9).reshape(3, 3)
VMAP = [np.rot90(_idx, r).reshape(9).tolist() for r in range(4)]

CHUNK_SEGS = []
for _ch in range(NCHUNK):
    segs = []
    g = _ch * 128
    while g < (_ch + 1) * 128:
        b_loc, c0 = g // C, g % C
        n = min((_ch + 1) * 128 - g, C - c0)
        segs.append((g - _ch * 128, n, b_loc, c0))
        g += n
    CHUNK_SEGS.append(segs)

KCFG = {0: 9, 1: 5, 2: 5, 3: 0}   # PE taps per rotation
AENG = {1: "vector", 2: "gpsimd", 3: "vector"}  # engine for product adds


def _build4(kcfg=None, aeng=None, do_stores=True):
    kcfg = kcfg or KCFG
    aeng = aeng or AENG
    nc = bacc.Bacc("TRN2", target_bir_lowering=False, debug=False, num_devices=NCORES)
    x_d = nc.dram_tensor("x", [ROWS, TH, TW], BF16, kind="ExternalInput").ap()
    w_d = nc.dram_tensor("w36", [ROWS, 36], F32, kind="ExternalInput").ap()
    wb_d = nc.dram_tensor("w36b", [ROWS, 36], BF16, kind="ExternalInput").ap()
    o_d = nc.dram_tensor("out", [BS * 4 * C, H, W], BF16, kind="ExternalOutput").ap()

    with tile.TileContext(nc) as tc, ExitStack() as ctx:
        xpool = ctx.enter_context(tc.tile_pool(name="xt", bufs=2))
        ppool = ctx.enter_context(tc.tile_pool(name="prod", bufs=2))
        opool = ctx.enter_context(tc.tile_pool(name="osb", bufs=5))
        wpool = ctx.enter_context(tc.tile_pool(name="wsb", bufs=2))
        dpool = ctx.enter_context(tc.tile_pool(name="diag", bufs=1))
        pspool = ctx.enter_context(tc.tile_pool(name="ps", bufs=8, space="PSUM"))

        def pe_rotation(r, k_pe, diag, xt, h0, osb):
            for s in range(NSUB):
                ps = pspool.tile([128, SUB, W], F32, tag="ps")
                for k in range(k_pe):
                    ti, tj = TAPS[k]
                    nc.tensor.matmul(
                        ps[:],
                        diag[:, r * 9 + ti * 3 + tj, :],
                        xt[:, h0 + s * SUB + ti : h0 + s * SUB + ti + SUB, tj : tj + W],
                        start=(k == 0),
                        stop=(k == k_pe - 1),
                    )
                nc.scalar.activation(
                    osb[:, s * SUB : (s + 1) * SUB, :],
                    ps[:],
                    mybir.ActivationFunctionType.Copy,
                )

        def store(ch, r, h0, osb):
            for si, (p0, n, b_loc, c0) in enumerate(CHUNK_SEGS[ch]):
                row0 = b_loc * 4 * C + r * C + c0
                dge = nc.scalar if (r + si) % 2 else nc.sync
                dge.dma_start(
                    o_d[row0 : row0 + n, h0 : h0 + HT, :], osb[p0 : p0 + n, :, :]
                )

        for ch in range(NCHUNK):
            g0 = ch * 128
            w_sb = wpool.tile([128, 36], F32, tag="wsb")
            nc.sync.dma_start(w_sb[:], w_d[g0 : g0 + 128, :])
            wb_sb = wpool.tile([128, 36], BF16, tag="wbsb")
            nc.sync.dma_start(wb_sb[:], wb_d[g0 : g0 + 128, :])
            diag = dpool.tile([128, 36, 128], BF16, tag="diag")
            nc.gpsimd.affine_select(
                out=diag[:],
                in_=wb_sb[:].broadcast_to([128, 36, 128]),
                compare_op=mybir.AluOpType.is_equal,
                fill=0.0,
                base=0,
                pattern=[[0, 36], [-1, 128]],
                channel_multiplier=1,
            )

            xt = xpool.tile([128, TH, TW], BF16, tag="xt")
            for li in range(4):
                r0 = li * (TH // 4)
                r1 = TH if li == 3 else r0 + TH // 4
                dge = [nc.sync, nc.scalar][li % 2]
                dge.dma_start(xt[:, r0:r1, :], x_d[g0 : g0 + 128, r0:r1, :])

            for ht in range(NHT):
                h0 = ht * HT
                osbs = {}
                for r in range(4):
                    osbs[r] = opool.tile([128, HT, W], BF16, tag="osb", name=f"osb{r}")
                    if kcfg[r] > 0:
                        pe_rotation(r, kcfg[r], diag, xt, h0, osbs[r])
                        if kcfg[r] == 9 and do_stores:
                            store(ch, r, h0, osbs[r])

                for g in range(NPG):
                    row0 = h0 + g * PH
                    ps_ = []
                    for m in range(9):
                        p = ppool.tile([128, PH + 2, TW], BF16, tag=f"p{m}")
                        nc.scalar.activation(
                            p[:],
                            xt[:, row0 : row0 + PH + 2, :],
                            mybir.ActivationFunctionType.Copy,
                            scale=w_sb[:, m : m + 1],
                        )
                        ps_.append(p)
                    for r in range(4):
                        kr = kcfg[r]
                        if kr == 9:
                            continue
                        eng = getattr(nc, aeng[r])
                        osl = osbs[r][:, g * PH : (g + 1) * PH, :]
                        ks = list(range(kr, 9))
                        if kr == 0:
                            ti0, tj0 = TAPS[ks[0]]
                            ti1, tj1 = TAPS[ks[1]]
                            m0 = VMAP[r][ti0 * 3 + tj0]
                            m1 = VMAP[r][ti1 * 3 + tj1]
                            eng.tensor_tensor(
                                out=osl,
                                in0=ps_[m0][:, ti0 : ti0 + PH, tj0 : tj0 + W],
                                in1=ps_[m1][:, ti1 : ti1 + PH, tj1 : tj1 + W],
                                op=mybir.AluOpType.add,
                            )
                            ks = ks[2:]
                        for k in ks:
                            ti, tj = TAPS[k]
                            m = VMAP[r][ti * 3 + tj]
                            eng.tensor_tensor(
                                out=osl,
                                in0=osl,
                                in1=ps_[m][:, ti : ti + PH, tj : tj + W],
                                op=mybir.AluOpType.add,
                            )
                if do_stores:
                    for r in range(4):
                        if kcfg[r] < 9:
                            store(ch, r, h0, osbs[r])

    nc.compile()
    return nc


def _make_w36(weight):
    w36 = np.zeros((C, 36), dtype=np.float32)
    base = weight[:, 0]
    for r in range(4):
        wr = np.rot90(base, r, axes=(1, 2))
        w36[:, r * 9 : (r + 1) * 9] = wr.reshape(C, 9)
    return np.tile(w36, (BS, 1))


def make_in_maps(x, weight):
    bf = mybir.dt.np(BF16)
    w36 = _make_w36(weight)
    w36b = w36.astype(bf)
    xp = np.zeros((B, C, TH, TW), dtype=bf)
    xp[:, :, 1 : H + 1, 1 : W + 1] = x.astype(bf)
    return [
        {
            "x": np.ascontiguousarray(xp[BS * k : BS * (k + 1)].reshape(ROWS, TH, TW)),
            "w36": w36,
            "w36b": w36b,
        }
        for k in range(NCORES)
    ]


from concourse.bass_utils import run_bass_kernel_spmd

_NC = None


def _get_nc():
    global _NC
    if _NC is None:
        _NC = _build4()
    return _NC


def kernel(x, weight):
    x = np.asarray(x, dtype=np.float32)
    weight = np.asarray(weight, dtype=np.float32)
    in_maps = make_in_maps(x, weight)
    nc = _get_nc()
    res = run_bass_kernel_spmd(nc, in_maps, list(range(NCORES))).results
    out = np.empty((B, 4 * C, H, W), dtype=np.float32)
    for k in range(NCORES):
        out[BS * k : BS * (k + 1)] = (
            res[k]["out"].astype(np.float32).reshape(BS, 4 * C, H, W)
        )
    return out



# revision 2
# speedup vs baseline: 15.1604x; 15.1604x over previous
"""bf16 C4 depthwise conv — all-PE dy-contraction kernel.

Each 32-channel group's padded input is staged as xrep[96, 128, 130]:
three row-shifted copies (vertical taps dy=0,1,2) stacked in partition
blocks, loaded with three parity-alternating DMAs on the SP HWDGE ring.
One bf16 stationary S_tj[96, 128] per horizontal tap then contracts all
three vertical taps AND emits all 4 rotations x 32 channels per matmul:
S_tj[(dy, c), (c*4 + r)] = rot90(w[c], r)[dy, tj].  Three matmuls
(tj=0,1,2) accumulate each [128, 4, 128] PSUM strip; two strips share a
[128, 8, 128] PSUM tile whose single drain (ACT/DVE alternating) casts
to bf16.  Outputs are packed partition-interleaved (p = c*4 + r) so
each fused per-rotation 1 MB store on the ACT ring reads a strided
partition set hitting all 16 SDMA engines.
"""

import numpy as np
from contextlib import ExitStack

from concourse import bacc, mybir, tile

B, C, H, W = 16, 192, 128, 128
NCORES = 8
BS = B // NCORES
ROWS = BS * C               # 384 (batch, channel) rows per core
G = 32                      # channels per group
NG = ROWS // G              # 12 groups
NGC = C // G                # 6 unique channel groups (stationaries repeat per batch)
TW = W + 2                  # 130
TH = H + 2                  # 130
SUB = 4                     # matmul rows (one PSUM bank, N=512)
PT = 8                      # psum tile rows (2 banks, one drain)
NSTRIP = H // PT            # 16 psum tiles per group

F32 = mybir.dt.float32
BF16 = mybir.dt.bfloat16

BLK = [0, 2, 1]             # dy -> xrep partition block (parity-alternating loads)


def _build(drain_pat=("act", "dve", "act", "dve", "act", "dve", "act", "act"),
           repeat=1):
    nc = bacc.Bacc("TRN2", target_bir_lowering=False, debug=False, num_devices=NCORES)
    x_d = nc.dram_tensor("x", [ROWS, TH, TW], BF16, kind="ExternalInput").ap()
    st_d = nc.dram_tensor("stat", [96, NGC, 3, 128], BF16, kind="ExternalInput").ap()
    o_d = nc.dram_tensor("out", [BS * 4 * C, H, W], BF16, kind="ExternalOutput").ap()

    with tile.TileContext(nc) as tc, ExitStack() as ctx:
        xpool = ctx.enter_context(tc.tile_pool(name="xrep", bufs=2))
        wpool = ctx.enter_context(tc.tile_pool(name="wst", bufs=1))
        opool = ctx.enter_context(tc.tile_pool(name="osb", bufs=2))
        pspool = ctx.enter_context(tc.tile_pool(name="ps", bufs=4, space="PSUM"))

        st_sb = wpool.tile([96, NGC, 3, 128], BF16, tag="st")
        nc.sync.dma_start(st_sb[:], st_d[:, :, :, :])

        di = 0
        for g in range(NG * repeat):
            g = g % NG
            b, gi = g // NGC, g % NGC
            xr = xpool.tile([96, H, TW], BF16, tag="xr")
            for dy in range(3):
                blk = BLK[dy]
                nc.sync.dma_start(
                    xr[32 * blk : 32 * (blk + 1), :, :],
                    x_d[G * g : G * (g + 1), dy : dy + H, :],
                )
            osb = opool.tile([128, H, W], BF16, tag="osb")
            for s in range(NSTRIP):
                r0 = s * PT
                ps = pspool.tile([128, PT, W], F32, tag="ps")
                for tj in range(3):
                    for m in range(PT // SUB):
                        nc.tensor.matmul(
                            ps[:, m * SUB : (m + 1) * SUB, :],
                            st_sb[:, gi, tj, :],
                            xr[:, r0 + m * SUB : r0 + m * SUB + SUB, tj : tj + W],
                            start=(tj == 0),
                            stop=(tj == 2),
                        )
                eng = drain_pat[di % len(drain_pat)]
                di += 1
                osl = osb[:, r0 : r0 + PT, :]
                if eng == "act":
                    nc.scalar.activation(
                        osl, ps[:], mybir.ActivationFunctionType.Copy
                    )
                else:
                    nc.vector.tensor_copy(osl, ps[:])
            for r in range(4):
                row0 = b * 4 * C + r * C + G * gi
                nc.scalar.dma_start(
                    o_d[row0 : row0 + G, :, :],
                    osb[r::4, :, :],
                )

    nc.compile()
    return nc


def _make_stat(weight):
    bf = mybir.dt.np(BF16)
    wb = weight[:, 0].astype(np.float32)          # [C, 3, 3]
    stat = np.zeros((96, NGC, 3, 128), dtype=np.float32)
    ar = np.arange(G)
    for r in range(4):
        wr = np.rot90(wb, r, axes=(1, 2))         # [C, 3, 3]
        for dy in range(3):
            for tj in range(3):
                vals = wr[:, dy, tj].reshape(NGC, G)   # [NGC, G]
                stat[BLK[dy] * G + ar, :, tj, ar * 4 + r] = vals.T
    return stat.astype(bf)


def make_in_maps(x, weight):
    bf = mybir.dt.np(BF16)
    stat = _make_stat(weight)
    xp = np.zeros((B, C, TH, TW), dtype=bf)
    xp[:, :, 1 : H + 1, 1 : W + 1] = x.astype(bf)
    return [
        {
            "x": np.ascontiguousarray(xp[BS * k : BS * (k + 1)].reshape(ROWS, TH, TW)),
            "stat": stat,
        }
        for k in range(NCORES)
    ]


from concourse.bass_utils import run_bass_kernel_spmd

_NC = None


def _get_nc():
    global _NC
    if _NC is None:
        _NC = _build()
    return _NC


def kernel(x, weight):
    x = np.asarray(x, dtype=np.float32)
    weight = np.asarray(weight, dtype=np.float32)
    in_maps = make_in_maps(x, weight)
    nc = _get_nc()
    res = run_bass_kernel_spmd(nc, in_maps, list(range(NCORES))).results
    out = np.empty((B, 4 * C, H, W), dtype=np.float32)
    for k in range(NCORES):
        out[BS * k : BS * (k + 1)] = (
            res[k]["out"].astype(np.float32).reshape(BS, 4 * C, H, W)
        )
    return out


# revision 3
# speedup vs baseline: 16.5058x; 1.0887x over previous
"""bf16 C4 depthwise conv — all-PE dy-contraction kernel.

Each 32-channel group's padded input is staged as xrep[96, 128, 130]:
three row-shifted copies (vertical taps dy=0,1,2) stacked in partition
blocks, loaded with three parity-alternating DMAs on the SP HWDGE ring.
One bf16 stationary S_tj[96, 128] per horizontal tap then contracts all
three vertical taps AND emits all 4 rotations x 32 channels per matmul:
S_tj[(dy, c), (c*4 + r)] = rot90(w[c], r)[dy, tj].  Three matmuls
(tj=0,1,2) accumulate each [128, 4, 128] PSUM strip; two strips share a
[128, 8, 128] PSUM tile whose single drain (ACT/DVE alternating) casts
to bf16.  Outputs are packed partition-interleaved (p = c*4 + r) so
each fused per-rotation 1 MB store on the ACT ring reads a strided
partition set hitting all 16 SDMA engines.
"""

import numpy as np
from contextlib import ExitStack

from concourse import bacc, mybir, tile

B, C, H, W = 16, 192, 128, 128
NCORES = 8
BS = B // NCORES
ROWS = BS * C               # 384 (batch, channel) rows per core
G = 32                      # channels per group
NG = ROWS // G              # 12 groups
NGC = C // G                # 6 unique channel groups (stationaries repeat per batch)
TW = W + 2                  # 130
TH = H + 2                  # 130
SUB = 4                     # matmul rows (one PSUM bank, N=512)
PT = 8                      # psum tile rows (2 banks, one drain)
NSTRIP = H // PT            # 16 psum tiles per group

F32 = mybir.dt.float32
BF16 = mybir.dt.bfloat16

BLK = [0, 2, 1]             # dy -> xrep partition block (parity-alternating loads)


def _build(drain_pat=("act", "dve", "act", "dve", "act", "dve", "act", "act"),
           repeat=1):
    nc = bacc.Bacc("TRN2", target_bir_lowering=False, debug=False, num_devices=NCORES)
    x_d = nc.dram_tensor("x", [ROWS, TH, TW], BF16, kind="ExternalInput").ap()
    st_d = nc.dram_tensor("stat", [96, NGC, 3, 128], BF16, kind="ExternalInput").ap()
    o_d = nc.dram_tensor("out", [BS * 4 * C, H, W], BF16, kind="ExternalOutput").ap()

    with tile.TileContext(nc) as tc, ExitStack() as ctx:
        xpool = ctx.enter_context(tc.tile_pool(name="xrep", bufs=3))
        wpool = ctx.enter_context(tc.tile_pool(name="wst", bufs=1))
        opool = ctx.enter_context(tc.tile_pool(name="osb", bufs=2))
        pspool = ctx.enter_context(tc.tile_pool(name="ps", bufs=4, space="PSUM"))

        st_sb = wpool.tile([96, NGC, 3, 128], BF16, tag="st")
        nc.sync.dma_start(st_sb[:], st_d[:, :, :, :])

        di = 0
        for g in range(NG * repeat):
            g = g % NG
            b, gi = g // NGC, g % NGC
            xr = xpool.tile([96, H, TW], BF16, tag="xr")
            for dy in range(3):
                blk = BLK[dy]
                nc.sync.dma_start(
                    xr[32 * blk : 32 * (blk + 1), :, :],
                    x_d[G * g : G * (g + 1), dy : dy + H, :],
                )
            osb = opool.tile([128, H, W], BF16, tag="osb")
            for s in range(NSTRIP):
                r0 = s * PT
                ps = pspool.tile([128, PT, W], F32, tag="ps")
                for tj in range(3):
                    for m in range(PT // SUB):
                        nc.tensor.matmul(
                            ps[:, m * SUB : (m + 1) * SUB, :],
                            st_sb[:, gi, tj, :],
                            xr[:, r0 + m * SUB : r0 + m * SUB + SUB, tj : tj + W],
                            start=(tj == 0),
                            stop=(tj == 2),
                        )
                eng = drain_pat[di % len(drain_pat)]
                di += 1
                osl = osb[:, r0 : r0 + PT, :]
                if eng == "act":
                    nc.scalar.activation(
                        osl, ps[:], mybir.ActivationFunctionType.Copy
                    )
                else:
                    nc.vector.tensor_copy(osl, ps[:])
            for r in range(4):
                row0 = b * 4 * C + r * C + G * gi
                nc.scalar.dma_start(
                    o_d[row0 : row0 + G, :, :],
                    osb[r::4, :, :],
                )

    nc.compile()
    return nc


def _make_stat(weight):
    bf = mybir.dt.np(BF16)
    wb = weight[:, 0].astype(np.float32)          # [C, 3, 3]
    stat = np.zeros((96, NGC, 3, 128), dtype=np.float32)
    ar = np.arange(G)
    for r in range(4):
        wr = np.rot90(wb, r, axes=(1, 2))         # [C, 3, 3]
        for dy in range(3):
            for tj in range(3):
                vals = wr[:, dy, tj].reshape(NGC, G)   # [NGC, G]
                stat[BLK[dy] * G + ar, :, tj, ar * 4 + r] = vals.T
    return stat.astype(bf)


def make_in_maps(x, weight):
    bf = mybir.dt.np(BF16)
    stat = _make_stat(weight)
    xp = np.zeros((B, C, TH, TW), dtype=bf)
    xp[:, :, 1 : H + 1, 1 : W + 1] = x.astype(bf)
    return [
        {
            "x": np.ascontiguousarray(xp[BS * k : BS * (k + 1)].reshape(ROWS, TH, TW)),
            "stat": stat,
        }
        for k in range(NCORES)
    ]


from concourse.bass_utils import run_bass_kernel_spmd

_NC = None


def _get_nc():
    global _NC
    if _NC is None:
        _NC = _build()
    return _NC


def kernel(x, weight):
    x = np.asarray(x, dtype=np.float32)
    weight = np.asarray(weight, dtype=np.float32)
    in_maps = make_in_maps(x, weight)
    nc = _get_nc()
    res = run_bass_kernel_spmd(nc, in_maps, list(range(NCORES))).results
    out = np.empty((B, 4 * C, H, W), dtype=np.float32)
    for k in range(NCORES):
        out[BS * k : BS * (k + 1)] = (
            res[k]["out"].astype(np.float32).reshape(BS, 4 * C, H, W)
        )
    return out
